# revision 14
# baseline (speedup 1.0000x reference)
"""Trainium2 Bass kernel for nn_DetuningEstimation.

Pipeline (per reference.py):
  STFT (2049 frames x 16384-pt rFFT, hann, hop 8192) -> |X|^2 -> log(100p+1)
  -> sum over time -> [8192] -> not-a-knot cubic spline interp at 8400 cent
  freqs -> subtract 101-tap mean smooth -> relu -> 84-tap dilated (x100) conv
  -> scores[100] -> argmax-50.

Distribution: 8 cores, ~256 frames each (core 7 gets 257; cores 0-6 process a
dummy all-zero 257th frame whose log-spectrum contribution is exactly 0).
AllReduce sums the per-core time-summed log-spectra; every core then runs the
tiny spline/conv/argmax tail redundantly (SPMD); host reads core 0's output.

FFT = two-stage Cooley-Tukey 128x128 as matmuls on the PE:
  stage A: per frame, Y[n1,k2] = sum_n2 xw[n1+128*n2] * W128^(n2*k2), k2=0..64
           (real input -> Hermitian in k2; Yi[0]=Yi[64]=0 so Y packs into 128
           cols: [Yr 0..64 | Yi 1..63])
  stage B: per output k2o, X[k2o+128*k1] = sum_n1 twiddle*Y via 2 matmuls with
           precomputed complex stationaries (twiddle+DFT folded, k1<64 only).
Window applied on-chip (hann tile multiply). Data fp16, accumulation fp32.
"""
import os

import numpy as np

import concourse.bass as bass
import concourse.bacc as bacc
import concourse.tile as tile
from concourse import mybir
from concourse.bass_utils import run_bass_kernel_spmd

F16 = mybir.dt.float16
F32 = mybir.dt.float32
BF16 = mybir.dt.bfloat16
ALU = mybir.AluOpType
ACTF = mybir.ActivationFunctionType

SR = 22050
N_FFT = 16384
HOP = N_FFT // 2
NF = N_FFT // 2
H = SR / N_FFT
NCORES = 8
FPC = 257            # frame slots per core (cores 0-6: last is a zero dummy)
NBLK = 130           # Audio0 blocks per core (blocks 0..128 real, 129 = jump)
ROWS = NBLK * 128    # 16640 rows of 128 samples in each core's DRAM buffer
NGRP = 16            # 16-frame groups covering frames 0..255
JFRAME = 256         # the jump frame (Audio0 block 129)

_CACHE = {}
LAST_EXEC_NS = None


# ----------------------------------------------------------------- host math
def _win_tile():
    w = (0.5 - 0.5 * np.cos(2.0 * np.pi * np.arange(N_FFT) / N_FFT))
    return w.reshape(128, 128)  # [n2, n1]


def _cs_matrix():
    """[128 n2, 128] : cols 0..64 = cos(2pi n2 k2/128), 65..127 = -sin, k2=1..63"""
    n2 = np.arange(128)[:, None]
    k2c = np.arange(65)[None, :]
    k2s = np.arange(1, 64)[None, :]
    C = np.cos(2 * np.pi * n2 * k2c / 128.0)
    S = -np.sin(2 * np.pi * n2 * k2s / 128.0)
    return np.concatenate([C, S], axis=1)  # [128, 128]


def _g_matrices():
    """Per pair m=0..63: 4 stacked stationaries [128 n1, 128] for matmuls
      xr_ps = G0^T@rhs0 + G1^T@rhs1   (Xr of k2o=m rows 0..63 | of 128-m rows 64..127)
      xi_ps = G2^T@rhs2 + G3^T@rhs3   (Xi halves)
    pair m>=1: rhs0=rhs2=Yr_col(m), rhs1=rhs3=Yi_col(64+m)
      G0=[Gr_m | Gr_{128-m}], G1=[-Gi_m | +Gi_{128-m}],
      G2=[Gi_m | Gi_{128-m}], G3=[Gr_m | -Gr_{128-m}]
    pair 0 (k2o 0 and 64): rhs0=rhs2=Yr_col(0), rhs1=rhs3=Yr_col(64)
      G0=[Gr_0 | 0], G1=[0 | Gr_64], G2=[Gi_0 | 0], G3=[0 | Gi_64]
    """
    n1 = np.arange(128)[:, None]
    k1 = np.arange(64)[None, :]
    E = np.exp(-2j * np.pi * n1 * k1 / 128.0)          # [128, 64]
    def G(k2o):
        return np.exp(-2j * np.pi * n1 * k2o / N_FFT) * E
    G4 = np.zeros((64, 4, 128, 128))
    g0 = G(0); g64 = G(64)
    G4[0, 0, :, 0:64] = g0.real
    G4[0, 1, :, 64:128] = g64.real
    G4[0, 2, :, 0:64] = g0.imag
    G4[0, 3, :, 64:128] = g64.imag
    for m in range(1, 64):
        ga = G(m); gb = G(128 - m)
        G4[m, 0, :, 0:64] = ga.real
        G4[m, 0, :, 64:128] = gb.real
        G4[m, 1, :, 0:64] = -ga.imag
        G4[m, 1, :, 64:128] = gb.imag
        G4[m, 2, :, 0:64] = ga.imag
        G4[m, 2, :, 64:128] = gb.imag
        G4[m, 3, :, 0:64] = ga.real
        G4[m, 3, :, 64:128] = -gb.real
    return G4


def _acc_bin_map():
    """bin index for acc entry (p, m): the device accumulates pair m with
    k2o=m rows 0..63 (k1=p) and k2o=128-m (m=0 -> 64) rows 64..127."""
    binmap = np.zeros((128, 64), np.int64)
    p = np.arange(64)
    for m in range(64):
        binmap[0:64, m] = m + 128 * p
        k2b = 64 if m == 0 else 128 - m
        binmap[64:128, m] = k2b + 128 * p
    return binmap  # flat acc index a = 64*p_acc + m  ->  bin binmap[p_acc, m]


def _tail_matrices(w_smooth, w_detect):
    """B = (I - S) @ A  (cent-interp minus smoothing), truncated to blocks.
    Returns (blocks dict (c,j)->[128,128] f64, const_vec = B@1 [8400],
             n_in_chunks, n_out_blocks)."""
    CENT = (440.0 * 2.0 ** ((np.arange(24, 108, 0.01) - 69.0) / 12.0)).astype(
        np.float64)
    NQ = CENT.shape[0]          # 8400
    h = np.float64(H)
    iq = np.clip(np.floor(CENT / h).astype(np.int64), 0, NF - 2)
    NI = int(iq.max()) + 2 + 64          # input support upper bound
    NI = ((NI + 127) // 128) * 128       # pad to chunk multiple

    # vectorized linear tail over unit inputs e_i, i < NI
    EY = np.eye(NF, NI)                        # [8192, NI] input columns
    d = (EY[:-2] - 2.0 * EY[1:-1] + EY[2:]) / (h * h)    # [8190, NI]
    M1 = d[0]; Mn2 = d[-1]
    rhs = 6.0 * d[1:-1]                        # [8188, NI]
    rhs[0] -= M1; rhs[-1] -= Mn2
    # Thomas with reference's precomputed coefficients
    M_UNK = NF - 4
    cp = np.empty(M_UNK); invd = np.empty(M_UNK)
    invd[0] = 0.25; cp[0] = 0.25
    for i in range(1, M_UNK):
        invd[i] = 1.0 / (4.0 - cp[i - 1])
        cp[i] = invd[i] if i < M_UNK - 1 else 0.0
    dp = np.zeros_like(rhs)
    prev = np.zeros(NI)
    for i in range(M_UNK):
        a = 0.0 if i == 0 else 1.0
        prev = (rhs[i] - a * prev) * invd[i]
        dp[i] = prev
    xs = np.zeros_like(dp)
    nxt = np.zeros(NI)
    for i in range(M_UNK - 1, -1, -1):
        nxt = dp[i] - cp[i] * nxt
        xs[i] = nxt
    M = np.concatenate([(2.0 * M1 - xs[0])[None], M1[None], xs,
                        Mn2[None], (2.0 * Mn2 - xs[-1])[None]], axis=0)  # [8192, NI]
    t = CENT - iq * h
    u = h - t
    Mi = M[iq]; Mi1 = M[iq + 1]; yi = EY[iq]; yi1 = EY[iq + 1]
    A = ((Mi * (u ** 3)[:, None] + Mi1 * (t ** 3)[:, None]) / (6.0 * h)
         + (yi - Mi * h * h / 6.0) * (u / h)[:, None]
         + (yi1 - Mi1 * h * h / 6.0) * (t / h)[:, None])   # [8400, NI]
    # smoothing S: smoothed = np.convolve(cent, w_smooth, 'same') (kernel flip)
    ws = np.asarray(w_smooth, np.float64)
    K = ws.shape[0]; half = (K - 1) // 2
    SA = np.zeros_like(A)
    # smoothed[q] = sum_m w[q - m + half] * cent[m]
    for kk in range(K):
        # contribution of w[kk]: m = q + half - kk
        shift = half - kk
        if shift >= 0:
            SA[:NQ - shift if shift else NQ] = SA[:NQ - shift if shift else NQ] \
                + ws[kk] * A[shift:]
        else:
            SA[-shift:] = SA[-shift:] + ws[kk] * A[:shift]
    B = A - SA
    const_vec = B.sum(axis=1)  # B @ 1 (valid since A reproduces constants,
                               # edges where S rows don't sum to 1 included)
    # permute input axis into the device acc layout: acc flat index
    # a = 64*p_acc + m holds bin binmap[p_acc, m]
    binmap = _acc_bin_map().reshape(-1)      # a -> bin
    nch = NF // 128
    nob = (NQ + 127) // 128
    blocks = {}
    for c in range(nch):
        bins = binmap[c * 128:(c + 1) * 128]
        sel = bins < NI
        if not sel.any():
            continue
        cols = np.zeros((NQ, 128))
        cols[:, sel] = B[:, bins[sel]]
        for j in range(nob):
            q0, q1 = j * 128, min((j + 1) * 128, NQ)
            blk = cols[q0:q1]
            if np.abs(blk).max() > 1e-7:
                P = np.zeros((128, 128))
                P[:, :q1 - q0] = blk.T   # lhsT layout [K=acc-chunk, M=out-q]
                blocks[(c, j)] = P
    return blocks, const_vec, nch, nob


# --------------------------------------------------------------- bass program
def _build(consts, fpc, dbg=False, sim1=False):
    nblk = (2 * fpc + 2 + 127) // 128 + (1 if fpc > 256 else 0)
    # audio buffer blocks: frames 0..fpc-2 from rows 0.., jump frame at block
    # jblk (only when fpc == 257); for reduced builds jump not used.
    full = fpc == FPC
    rows = ROWS if full else ((64 * (fpc + 1) + 127) // 128) * 128
    nb_audio = rows // 128

    blocks, _, nch, nob = consts["tail_shape"]
    nblocks_b = len(consts["b_keys"])

    nc = bacc.Bacc("TRN2", target_bir_lowering=False, debug=False,
                   enable_asserts=False,
                   num_devices=1 if sim1 else NCORES)
    audio = nc.dram_tensor("audio", [rows * 128], F16, kind="ExternalInput").ap()
    cs_d = nc.dram_tensor("cs", [128, 128], F16, kind="ExternalInput").ap()
    win_d = nc.dram_tensor("win", [128, 128], F16, kind="ExternalInput").ap()
    g4_d = nc.dram_tensor("g4", [128, 64 * 4 * 128], F16, kind="ExternalInput").ap()
    bb_d = nc.dram_tensor("bb", [128, nblocks_b * 128], F16, kind="ExternalInput").ap()
    cv_d = nc.dram_tensor("cv", [128, nob], F32, kind="ExternalInput").ap()
    wd_d = nc.dram_tensor("wd", [128, 2], F16, kind="ExternalInput").ap()  # rows 0..83 used
    out_scores = nc.dram_tensor("out_scores", [1, 100], F32, kind="ExternalOutput").ap()
    out_det = nc.dram_tensor("out_det", [1, 1], mybir.dt.int32, kind="ExternalOutput").ap()
    if dbg:
        out_spec = nc.dram_tensor("out_spec", [NF], F32, kind="ExternalOutput").ap()
        out_acc = nc.dram_tensor("out_acc", [128, 64], F32, kind="ExternalOutput").ap()

    audio_r = audio.rearrange("(b p j) -> b p j", p=128, j=128)  # blocks

    with tile.TileContext(nc) as tc:
        with tc.tile_pool(name="const", bufs=1) as constp, \
             tc.tile_pool(name="gpool", bufs=1) as gpool, \
             tc.tile_pool(name="ypool", bufs=1) as ypool, \
             tc.tile_pool(name="accp", bufs=1) as accp, \
             tc.tile_pool(name="dram", bufs=1, space="DRAM") as dram:

            cs = constp.tile([128, 128], F16)
            w0 = constp.tile([128, 128], F16)
            nc.sync.dma_start(cs[:], cs_d[:])
            nc.sync.dma_start(w0[:], win_d[:])
            g4 = gpool.tile([128, 64 * 4 * 128], F16, tag="g4")
            nc.sync.dma_start(g4[:], g4_d[:])
            bb = gpool.tile([128, nblocks_b * 128], F16, tag="bb")
            nc.sync.dma_start(bb[:], bb_d[:])

            yb = ypool.tile([128, 128 * fpc], F16)    # packed Y, k2-major
            yb3 = yb[:].rearrange("p (k l) -> p l k", k=128, l=fpc)
            acc = accp.tile([128, 64], F32)

            # ---------------- phase A: window + stage-A FFT + Y pack
            ngrp = (fpc - 1) // 16 if full else max(fpc // 16, 0)
            n_grp_frames = ngrp * 16

            with tc.tile_pool(name="raw", bufs=3) as rawp, \
                 tc.tile_pool(name="winb", bufs=3) as winp, \
                 tc.tile_pool(name="yps", bufs=4, space="PSUM") as yps:

                def flush_pair(pslot, l0, n):
                    # copy n frames' Y from psum [128, n*128] to yb (cast f16)
                    src = pslot[:].rearrange("p (q k) -> p q k", q=4)[:, 0:n, :]
                    dst = yb3[:, l0:l0 + n, :]
                    nc.vector.tensor_copy(dst[:], src[:])

                frames_done = 0
                for g in range(ngrp):
                    b0 = 8 * g
                    rawE = rawp.tile([128, 1024], F16, tag="rawE")
                    rawO = rawp.tile([128, 1024], F16, tag="rawO")
                    # even source rows 128*b0.., odd source rows 128*b0+64..
                    nc.sync.dma_start(
                        rawE[:].rearrange("p (b j) -> p b j", b=8),
                        audio_r[b0:b0 + 8].rearrange("b p j -> p b j"))
                    nc.sync.dma_start(
                        rawO[:].rearrange("p (b j) -> p b j", b=8),
                        audio[b0 * 16384 + 64 * 128:(b0 + 8) * 16384 + 64 * 128]
                        .rearrange("(b p j) -> p b j", p=128, j=128))
                    winE = winp.tile([128, 1024], F16, tag="winE")
                    winO = winp.tile([128, 1024], F16, tag="winO")
                    w0b = w0[:].rearrange("p (u j) -> p u j", u=1) \
                        .to_broadcast((128, 8, 128))
                    nc.gpsimd.tensor_tensor(
                        winE[:].rearrange("p (b j) -> p b j", b=8),
                        rawE[:].rearrange("p (b j) -> p b j", b=8), w0b, ALU.mult)
                    nc.gpsimd.tensor_tensor(
                        winO[:].rearrange("p (b j) -> p b j", b=8),
                        rawO[:].rearrange("p (b j) -> p b j", b=8), w0b, ALU.mult)
                    for e in range(8):
                        for par, wt in ((0, winE), (1, winO)):
                            l = 16 * g + 2 * e + par
                            slot = l % 4
                            if slot == 0:
                                ps = yps.tile([128, 512], F32, tag="ypair")
                            nc.tensor.matmul(
                                ps[:, slot * 128:(slot + 1) * 128],
                                wt[:, e * 128:(e + 1) * 128], cs[:],
                                start=True, stop=True)
                            if slot == 3:
                                flush_pair(ps, l - 3, 4)
                    frames_done = 16 * ngrp
                # leftover frames (reduced builds) + jump frame (full build)
                for l in range(frames_done, fpc):
                    if full and l == JFRAME:
                        blk = audio_r[129:130].opt()
                    else:
                        b = l // 2
                        if l % 2 == 0:
                            blk = audio_r[b]
                        else:
                            blk = audio[b * 16384 + 64 * 128:
                                        (b + 1) * 16384 + 64 * 128] \
                                .rearrange("(p j) -> p j", p=128)
                    rawT = rawp.tile([128, 128], F16, tag="rawT")
                    nc.sync.dma_start(rawT[:], blk)
                    winT = winp.tile([128, 128], F16, tag="winT")
                    nc.gpsimd.tensor_tensor(winT[:], rawT[:], w0[:], ALU.mult)
                    ps = yps.tile([128, 512], F32, tag="ypair")
                    nc.tensor.matmul(ps[:, 0:128], winT[:], cs[:],
                                     start=True, stop=True)
                    flush_pair(ps, l, 1)

            # ---------------- phase B: stage-B FFT + power + log-accum
            with tc.tile_pool(name="xps", bufs=3, space="PSUM") as xps, \
                 tc.tile_pool(name="sqp", bufs=4) as sqp, \
                 tc.tile_pool(name="pp", bufs=3) as pp, \
                 tc.tile_pool(name="lnout", bufs=2) as lnp:
                for m in range(64):
                    if m == 0:
                        r0 = r2 = 0
                        r1 = r3 = 64
                    else:
                        r0 = r2 = m
                        r1 = r3 = 64 + m
                    gbase = 4 * m * 128
                    xx = xps.tile([128, 1024], F32, tag="XX")  # 2 banks: xr @0, xi @512
                    xr = xx[:, 0:fpc]
                    xi = xx[:, 512:512 + fpc]
                    nc.tensor.matmul(xr, g4[:, gbase:gbase + 128],
                                     yb[:, r0 * fpc:(r0 + 1) * fpc],
                                     start=True, stop=False)
                    nc.tensor.matmul(xr, g4[:, gbase + 128:gbase + 256],
                                     yb[:, r1 * fpc:(r1 + 1) * fpc],
                                     start=False, stop=True)
                    nc.tensor.matmul(xi, g4[:, gbase + 256:gbase + 384],
                                     yb[:, r2 * fpc:(r2 + 1) * fpc],
                                     start=True, stop=False)
                    nc.tensor.matmul(xi, g4[:, gbase + 384:gbase + 512],
                                     yb[:, r3 * fpc:(r3 + 1) * fpc],
                                     start=False, stop=True)
                    sq = sqp.tile([128, 2 * fpc], BF16, tag="sq")
                    nc.scalar.square(
                        sq[:].rearrange("p (t f) -> p t f", t=2),
                        xx[:].rearrange("p (t f) -> p t f", t=2)[:, :, 0:fpc])
                    ptile = pp.tile([128, fpc], BF16, tag="P")
                    nc.gpsimd.tensor_tensor(ptile[:], sq[:, 0:fpc],
                                            sq[:, fpc:2 * fpc], ALU.add)
                    lnt = lnp.tile([128, fpc], BF16, tag="ln")
                    nc.scalar.activation(lnt[:], ptile[:], ACTF.Ln,
                                         bias=1.0, scale=100.0,
                                         accum_out=acc[:, m:m + 1])

            # ---------------- collective: acc -> canonical spec, AllReduce
            spec_in = dram.tile([NF], F32, tag="spec_in")
            spec_rd = dram.tile([NF], F32, tag="spec_rd")
            nc.sync.dma_start(
                spec_in[:].rearrange("(p m) -> p m", p=128), acc[:])
            if sim1:
                nc.sync.dma_start(spec_rd[:], spec_in[:])
            else:
                nc.gpsimd.collective_compute(
                    "AllReduce", ALU.add, replica_groups=[list(range(NCORES))],
                    ins=[spec_in.opt()], outs=[spec_rd.opt()])
            if dbg:
                nc.sync.dma_start(out_spec[:], spec_rd[:])
                nc.sync.dma_start(out_acc[:], acc[:])

            # ---------------- tail (redundant on every core)
            with tc.tile_pool(name="tailp", bufs=1) as tp, \
                 tc.tile_pool(name="tps", bufs=2, space="PSUM") as tps:
                ysb = tp.tile([128, 64], F32)
                nc.sync.dma_start(ysb[:], spec_rd[:]
                                  .rearrange("(c p) -> p c", p=128))
                # mean over 8192
                rsum = tp.tile([128, 1], F32)
                nc.vector.tensor_reduce(rsum[:], ysb[:], mybir.AxisListType.X, ALU.add)
                ones = tp.tile([128, 1], F32)
                nc.vector.memset(ones[:], 1.0 / 8192.0)
                mean_ps = tps.tile([1, 1], F32, tag="mean")
                nc.tensor.matmul(mean_ps[:], ones[:], rsum[:],
                                 start=True, stop=True)
                mean_sb = tp.tile([1, 1], F32)
                nc.scalar.copy(mean_sb[:], mean_ps[:])
                ybc = tp.tile([128, 1], F32)
                nc.gpsimd.partition_broadcast(ybc[:], mean_sb[:])
                yp = tp.tile([128, 64], F32)
                nc.vector.tensor_scalar(yp[:], ysb[:], ybc[:], None, ALU.subtract)
                yhi = tp.tile([128, 64], F16)
                nc.vector.tensor_copy(yhi[:], yp[:])
                ylo32 = tp.tile([128, 64], F32)
                nc.vector.tensor_tensor(ylo32[:], yp[:], yhi[:], ALU.subtract)
                ylo = tp.tile([128, 64], F16)
                nc.vector.tensor_copy(ylo[:], ylo32[:])

                cent_ps = tps.tile([128, nob], F32, tag="cent")
                b_keys = consts["b_keys"]
                by_j = {}
                for idx, (c, j) in enumerate(b_keys):
                    by_j.setdefault(j, []).append((c, idx))
                for j in range(nob):
                    lst = by_j.get(j, [])
                    seq = []
                    for c, idx in lst:
                        seq.append((idx, yhi, c))
                    for c, idx in lst:
                        seq.append((idx, ylo, c))
                    for s, (idx, ysrc, c) in enumerate(seq):
                        nc.tensor.matmul(
                            cent_ps[:, j:j + 1],
                            bb[:, idx * 128:(idx + 1) * 128],
                            ysrc[:, c:c + 1],
                            start=(s == 0), stop=(s == len(seq) - 1))
                cvt = tp.tile([128, nob], F32)
                nc.sync.dma_start(cvt[:], cv_d[:])
                cent = tp.tile([128, nob], F32)
                nc.vector.scalar_tensor_tensor(
                    cent[:], cvt[:], ybc[:], cent_ps[:], ALU.mult, ALU.add)
                filt = tp.tile([128, nob], F16)
                nc.scalar.activation(filt[:], cent[:], ACTF.Relu)
                # reshape via DRAM bounce: [128, nob] (q=128j+p) -> [84, 100]
                filt_d = dram.tile([8448], F16, tag="filt")
                nc.sync.dma_start(
                    filt_d[:].rearrange("(j p) -> p j", p=128)[:, 0:nob],
                    filt[:])
                f2 = tp.tile([84, 100], F16)
                nc.sync.dma_start(f2[:], filt_d[0:8400]
                                  .rearrange("(k q) -> k q", k=84))
                wdt = tp.tile([128, 2], F16)
                nc.sync.dma_start(wdt[:], wd_d[:])
                sc0 = tps.tile([1, 100], F32, tag="sc0")
                sc1 = tps.tile([1, 100], F32, tag="sc1")
                nc.tensor.matmul(sc0[:], wdt[0:84, 0:1], f2[:],
                                 start=True, stop=True)
                nc.tensor.matmul(sc1[:], wdt[0:84, 1:2], f2[:],
                                 start=True, stop=True)
                scores = tp.tile([1, 100], F32)
                nc.vector.tensor_copy(scores[:, 50:100], sc0[:, 0:50])
                nc.scalar.copy(scores[:, 0:50], sc1[:, 50:100])
                nc.sync.dma_start(out_scores[:], scores[:])
                mv = tp.tile([1, 8], F32)
                mi = tp.tile([1, 8], mybir.dt.uint32)
                nc.vector.max_with_indices(mv[:], mi[:], scores[:])
                mif = tp.tile([1, 1], F32)
                nc.vector.tensor_copy(mif[:], mi[:, 0:1])
                det_f = tp.tile([1, 1], F32)
                nc.vector.tensor_scalar(det_f[:], mif[:], 50.0, None, ALU.subtract)
                det_i = tp.tile([1, 1], mybir.dt.int32)
                nc.vector.tensor_copy(det_i[:], det_f[:])
                nc.sync.dma_start(out_det[:], det_i[:])

    nc.compile()
    return nc


# ----------------------------------------------------------------- host side
def _consts(w_smooth, w_detect):
    blocks, const_vec, nch, nob = _tail_matrices(w_smooth, w_detect)
    b_keys = sorted(blocks.keys(), key=lambda cj: (cj[1], cj[0]))
    bb = np.zeros((128, len(b_keys) * 128), np.float16)
    for i, k in enumerate(b_keys):
        bb[:, i * 128:(i + 1) * 128] = blocks[k].astype(np.float16)
    cv = np.zeros((128, nob), np.float32)
    for j in range(nob):
        q0, q1 = j * 128, min((j + 1) * 128, 8400)
        cv[0:q1 - q0, j] = const_vec[q0:q1].astype(np.float32)
    wd = np.zeros((128, 2), np.float16)
    wd[0:84, 0] = np.asarray(w_detect, np.float16)
    wd[0:83, 1] = np.asarray(w_detect[1:84], np.float16)
    G4 = _g_matrices()          # [64, 4, 128, 128]
    g4 = np.ascontiguousarray(
        G4.transpose(2, 0, 1, 3).reshape(128, 64 * 4 * 128)).astype(np.float16)
    return {
        "cs": _cs_matrix().astype(np.float16),
        "win": _win_tile().astype(np.float16),
        "g4": g4, "bb": bb, "cv": cv, "wd": wd,
        "b_keys": b_keys, "tail_shape": (blocks, const_vec, nch, nob),
    }


def _audio_buffers(audio):
    x = np.pad(np.asarray(audio, np.float32), N_FFT // 2, mode="reflect")
    rows = x.reshape(-1, 128)   # [131200, 128]
    bufs = []
    for c in range(NCORES):
        buf = np.zeros((ROWS, 128), np.float16)
        r0 = 64 * 256 * c
        buf[0:16448] = rows[r0:r0 + 16448].astype(np.float16)
        if c == NCORES - 1:
            buf[16512:16640] = rows[131072:131200].astype(np.float16)
        bufs.append(buf.reshape(-1))
    return bufs


def kernel(audio, w_smooth, w_detect):
    audio = np.asarray(audio)
    w_smooth = np.asarray(w_smooth, np.float32)
    w_detect = np.asarray(w_detect, np.float32)
    key = ("prog", w_smooth.tobytes(), w_detect.tobytes())
    if key not in _CACHE:
        consts = _consts(w_smooth, w_detect)
        nc = _build(consts, FPC)
        _CACHE[key] = (nc, consts)
    nc, consts = _CACHE[key]
    bufs = _audio_buffers(audio)
    cmaps = {k: consts[k] for k in ("cs", "win", "g4", "bb", "cv", "wd")}
    in_maps = [dict(audio=bufs[c], **cmaps) for c in range(NCORES)]
    trace = os.environ.get("BASSK_TRACE") == "1"
    res = run_bass_kernel_spmd(nc, in_maps, core_ids=list(range(NCORES)),
                               trace=trace)
    global LAST_EXEC_NS
    LAST_EXEC_NS = res.exec_time_ns
    out = res.results[0]
    scores = out["out_scores"].reshape(100).astype(np.float32)
    det = np.int32(out["out_det"].reshape(())[()])
    return np.asarray(det, np.int32), scores
